# revision 57
# baseline (speedup 1.0000x reference)
"""Trainium2 Bass kernel for CrossStockAttention (sparse top-40 attention).

Strategy (8 NeuronCores, zero inter-core communication):
  - Data-parallel: core = (batch b, query-half). Each core owns 1024 queries of
    one batch and all of that batch's valid keys (compacted, padded to 128).
  - Host-side marshalling: queries permuted valid-first; keys compacted to
    valid-only. Permutation undone on the host after the gather.
  - Ranking trick: cosine top-k per query row is invariant to the positive
    per-row scale 1/|x_q|, so only KEYS are normalized (sim = x_q . x_k/|x_k|).
  - Exact top-40 threshold per valid query: 5 rounds of DVE max8+match_replace
    give the 40 largest values; t40 = min of the last round's 8. The mask is
    (sim >= t40) -- exact fp32, tie-free for random data. Invalid-query rows
    get t40 = -1e30 (attend to every valid key; padded keys are harmless since
    their V rows and denominator gates are zero).
  - Attention in transposed score layout S_T[k, q]: the softmax denominator is
    an extra all-ones column appended to V (gated by the valid-key indicator),
    and attn@V needs no transposes.
  - Division by the softmax denominator is folded into the per-head output
    projection merge via scalar_tensor_tensor in token-major layout.
  - Engine balance: LayerNorm runs on the (otherwise idle) GpSimd engine; exp
    on Scalar; top-k scan on Vector; matmuls dense on PE. Scores+exp for heads
    0-1 are emitted during the top-k phase so PE/ACT overlap the DVE scan.
"""

import math
import numpy as np
import ml_dtypes

import concourse.bass as bass
import concourse.mybir as mybir
from concourse.tile import TileContext
from concourse import bass_utils, bacc

B, N, D, H, TOPK = 4, 2048, 512, 8, 40
DH = D // H
NQ = N // 2          # queries per core
QT = NQ // 128       # query tiles (8)
DC = D // 128        # feature chunks (4)
F32 = mybir.dt.float32
BF16 = mybir.dt.bfloat16
AF = mybir.ActivationFunctionType
ALU = mybir.AluOpType


def _chunk3(x, p=128):
    """[A*p, F] -> [p, A, F] (partition-major chunking along dim0)."""
    a = x.shape[0] // p
    return np.ascontiguousarray(x.reshape(a, p, -1).transpose(1, 0, 2))


def _nchunks(total, step=512):
    out = []
    o = 0
    while o < total:
        out.append((o, min(step, total - o)))
        o += step
    return out


def build_nc(KV, VT, KS):
    """Build the single-core program (SPMD across 8 cores, data differs).

    KS = number of key columns that can hold real keys (max valid count,
    rounded up to 8). Columns beyond KS are -1e9 bias with zero V rows, so
    the top-k scan and the sim products skip them.
    """
    KT = KV // 128       # key tiles
    NV = VT * 128        # padded valid-query count
    kv_ch = _nchunks(KV)
    v_ch = _nchunks(H * 65)

    nc = bacc.Bacc("TRN2", target_bir_lowering=False, debug=False, num_devices=8)

    di = lambda name, shape: nc.dram_tensor(name, shape, F32, kind="ExternalInput")
    db = lambda name, shape: nc.dram_tensor(name, shape, BF16, kind="ExternalInput")
    # group 1: gates the projections (small, bf16) -- loaded first
    wq_t_d = db("wq_t", [128, DC, D])
    bq_col_d = di("bq_col", [128, DC])
    bk_col_d = di("bk_col", [128, DC])
    xq_tb_d = db("xq_tb", [128, DC, NQ])
    wk_t_d = db("wk_t", [128, DC, D])
    wv_t_d = db("wv_t", [128, DC, H * 65])
    xk_tb_d = db("xk_tb", [128, DC, KV])
    validk_b_d = db("validk_b", [1, KV])
    extra_rhs_d = db("extra_rhs", [1, H * 65])
    # group 2: gates sim + topk (nrmk = x_k/|x_k| precomputed on host).
    # sim MUST be fp32: the top-40 boundary decisions have gaps down to
    # ~1e-6 and must match the fp32 reference.
    nrmk_t_d = di("nrmk_t", [128, DC, KV])
    xq_t_d = di("xq_t", [128, DC, NQ])
    simbias_b_d = db("simbias_b", [1, KV])
    onesb_d = db("onesb", [1, 128])
    invq_d = di("invq", [128, VT])
    identb_d = db("identb", [128, 128])
    # group 3: needed late (attention tail / LN)
    wo_t_d = db("wo_t", [64, H, D])
    xq_d = di("xq", [128, QT, D])
    lnd_scratch = [nc.dram_tensor(f"lnd_scratch{i}", [NQ], F32, kind="Internal")
                   for i in range(2)]
    out_d = nc.dram_tensor("out", [128, QT, D], F32, kind="ExternalOutput")

    with TileContext(nc) as tc:
        with (
            tc.tile_pool(name="consts", bufs=1) as consts,
            tc.tile_pool(name="bigbuf", bufs=1) as bigbuf,
            tc.tile_pool(name="stream", bufs=2) as stream,
            tc.tile_pool(name="simp", bufs=2) as simp,
            tc.tile_pool(name="pmm", bufs=2, space="PSUM") as pmm,
            tc.tile_pool(name="psc", bufs=2, space="PSUM") as psc,
            tc.tile_pool(name="pcp", bufs=2, space="PSUM") as pcp,
            tc.tile_pool(name="small", bufs=1) as small,
        ):
            def load(dram, shape, dtype=F32, pool=consts, tag=None):
                t = pool.tile(shape, dtype, tag=tag or dram.name)
                nc.sync.dma_start(t[:], dram.ap())
                return t

            def load_chunked(dram, shape, dtype, tag, pool=bigbuf):
                t = pool.tile(shape, dtype, tag=tag)
                for c in range(shape[1]):
                    nc.sync.dma_start(t[:, c, :], dram.ap()[:, c, :])
                return t

            # ---- loads, ordered by first use: Q-proj -> sim -> K/V proj ----
            wq_t = load(wq_t_d, [128, DC, D], BF16)
            bq_col = load(bq_col_d, [128, DC])
            bk_col = load(bk_col_d, [128, DC])
            xq_tb = load_chunked(xq_tb_d, [128, DC, NQ], BF16, "xq_tb")
            nrmk_t = load_chunked(nrmk_t_d, [128, DC, KV], F32, "nrmk")
            xq_t = load_chunked(xq_t_d, [128, DC, NQ], F32, "xq_t")
            simbias_b = load(simbias_b_d, [1, KV], BF16)
            onesb = load(onesb_d, [1, 128], BF16)
            invq = load(invq_d, [128, VT])
            identb = load(identb_d, [128, 128], BF16)
            wk_t = load(wk_t_d, [128, DC, D], BF16)
            xk_tb = load_chunked(xk_tb_d, [128, DC, KV], BF16, "xk_tb")
            wv_t = load(wv_t_d, [128, DC, H * 65], BF16)
            validk_b = load(validk_b_d, [1, KV], BF16)
            extra_rhs = load(extra_rhs_d, [1, H * 65], BF16)
            wo_t = load(wo_t_d, [64, H, D], BF16)

            eps_col = consts.tile([128, 1], F32)
            nc.vector.memset(eps_col[:], 1.0e-5)

            # ---- attention scores (emitted per head; h0/h1 overlap top-k) ----
            att = bigbuf.tile([128, QT, D], F32, tag="att")

            def emit_scores(h):
                hp = (h % 2) * 64
                hc = h // 2
                # three slots deepen the scores->exp->tail pipeline; slot
                # "nrmk" is the sim-input buffer, dead once head 2 starts
                expm = bigbuf.tile([128, KT, NQ], BF16,
                                   tag=["expm0", "expm1", "nrmk"][h % 3])
                for kt in range(KT):
                    ks = slice(kt * 128, (kt + 1) * 128)
                    ps = psc.tile([128, 1024], F32, tag="sc")
                    for (o, n) in _nchunks(NQ):
                        nc.tensor.matmul(
                            ps[:, o:o + n],
                            kt_sb[hp:hp + 64, hc, ks],
                            qt_sb[hp:hp + 64, hc, o:o + n],
                            start=True, stop=True)
                    nc.scalar.activation(expm[:, kt, :], ps[:],
                                         AF.Exp, scale=1.0 / math.sqrt(DH))
                return expm

            # ---- sim (bf16 hi/lo) + exact top-40 threshold mask per q tile ----
            scope_sim = nc.enter_named_scope("p_simtopk", False)
            maskT = bigbuf.tile([128, KT, NV], BF16, tag="maskT")
            work = small.tile([128, KV], F32, tag="work")
            scr8 = small.tile([128, 8], F32, tag="scr8")
            t40 = small.tile([128, 1], F32, tag="t40")

            def emit_sim_topk(vt):
                qs = slice(vt * 128, (vt + 1) * 128)
                sim = simp.tile([128, KV], F32, tag="sim")
                for (o, n) in kv_ch:
                    ps = pmm.tile([128, 512], F32, tag="mm")
                    nc.tensor.matmul(ps[:, :n], onesb[:],
                                     simbias_b[:, o:o + n], start=True, stop=False)
                    nr = max(0, min(KS - o, n))  # cols that can hold real keys
                    for c in range(DC):
                        nc.tensor.matmul(ps[:, :nr], xq_t[:, c, qs],
                                         nrmk_t[:, c, o:o + nr],
                                         start=False, stop=(c == DC - 1))
                    nc.scalar.copy(sim[:, o:o + n], ps[:, :n])
                # round 1 reads sim directly; match_replace writes the work
                # copy; the final round needs no replace (only its max8)
                nc.vector.max(scr8[:], sim[:, 0:KS])
                nc.vector.match_replace(work[:, 0:KS], scr8[:], sim[:, 0:KS],
                                        -1.0e9)
                for r in range(TOPK // 8 - 2):
                    nc.vector.max(scr8[:], work[:, 0:KS])
                    nc.vector.match_replace(work[:, 0:KS], scr8[:],
                                            work[:, 0:KS], -1.0e9)
                nc.vector.max(scr8[:], work[:, 0:KS])
                # t40 = 40th-largest; invalid-query rows attend to everything
                nc.vector.tensor_reduce(t40[:], scr8[:], axis=mybir.AxisListType.X,
                                        op=ALU.min)
                nc.vector.scalar_tensor_tensor(
                    t40[:], invq[:, vt:vt + 1], -1.0e30, t40[:],
                    op0=ALU.mult, op1=ALU.add)
                maskv = simp.tile([128, KV], BF16, tag="maskv")
                nc.vector.tensor_scalar(maskv[:], sim[:], t40[:], None,
                                        op0=ALU.is_ge)
                # transpose this tile's mask into maskT[k, q]
                for kt in range(KT):
                    pt = pcp.tile([128, 128], BF16, tag="cp")
                    nc.tensor.transpose(
                        pt[:], maskv[:, kt * 128:(kt + 1) * 128], identb[:])
                    nc.vector.tensor_copy(maskT[:, kt, qs], pt[:])

            # ---- Q projection first (its inputs land first), then the sim
            # tiles that gate the serial Vector top-k chain ----
            qt_sb = bigbuf.tile([128, DC, NQ], BF16, tag="qt")
            for dot in range(DC):
                for (o, n) in _nchunks(NQ):
                    ps = pmm.tile([128, 512], F32, tag="mm")
                    for c in range(DC):
                        nc.tensor.matmul(
                            ps[:, :n],
                            wq_t[:, c, dot * 128:(dot + 1) * 128],
                            xq_tb[:, c, o:o + n],
                            start=(c == 0), stop=(c == DC - 1))
                    nc.scalar.activation(qt_sb[:, dot, o:o + n], ps[:, :n],
                                         AF.Identity, bias=bq_col[:, dot:dot + 1])

            emit_sim_topk(0)
            emit_sim_topk(1)

            kt_sb = bigbuf.tile([128, DC, KV], BF16, tag="kt")
            for dot in range(DC):
                for (o, n) in kv_ch:
                    ps = pmm.tile([128, 512], F32, tag="mm")
                    for c in range(DC):
                        nc.tensor.matmul(
                            ps[:, :n],
                            wk_t[:, c, dot * 128:(dot + 1) * 128],
                            xk_tb[:, c, o:o + n],
                            start=(c == 0), stop=(c == DC - 1))
                    nc.scalar.activation(kt_sb[:, dot, o:o + n], ps[:, :n],
                                         AF.Identity, bias=bk_col[:, dot:dot + 1])

            emit_sim_topk(2)
            emit_sim_topk(3)
            emit_sim_topk(4)
            expm_pre = [emit_scores(0)]

            vaug = bigbuf.tile([128, KT, H * 65], BF16, tag="vaug")
            for kt in range(KT):
                ks = slice(kt * 128, (kt + 1) * 128)
                for (o, n) in v_ch:
                    ps = pmm.tile([128, 512], F32, tag="mm")
                    nc.tensor.matmul(ps[:, :n], validk_b[0:1, ks],
                                     extra_rhs[:, o:o + n], start=True, stop=False)
                    for c in range(DC):
                        nc.tensor.matmul(
                            ps[:, :n], xk_tb[:, c, ks],
                            wv_t[:, c, o:o + n],
                            start=False, stop=(c == DC - 1))
                    nc.scalar.copy(vaug[:, kt, o:o + n], ps[:, :n])

            expm_pre.append(emit_scores(1))
            nc.leave_named_scope("p_simtopk", scope_sim[0], False)

            # ---- attention tails, software-pipelined across heads ----
            scope_att = nc.enter_named_scope("p_attn", False)

            def emit_tail(h, expm):
                # mask multiply: GpSimd takes some heads to relieve Vector.
                # Head 0 splits by ctx chunk on GpSimd so its first ctx
                # matmul starts while Vector is still scanning sim tile 4.
                eng = nc.gpsimd if h in (0, 5, 6, 7) else nc.vector
                if h != 0:
                    for kt in range(KT):
                        eng.tensor_tensor(expm[:, kt, 0:NV], expm[:, kt, 0:NV],
                                          maskT[:, kt, :], op=ALU.mult)
                ctx_h = stream.tile([64, NQ], BF16, tag="ctx")
                lnd = small.tile([1, NQ], F32, tag="lnd")
                scr = lnd_scratch[h % 2]
                for (o, n) in _nchunks(NQ):
                    if h == 0:
                        mo, mn = o, max(0, min(NV, o + n) - o)
                        for kt in range(KT):
                            eng.tensor_tensor(expm[:, kt, mo:mo + mn],
                                              expm[:, kt, mo:mo + mn],
                                              maskT[:, kt, mo:mo + mn],
                                              op=ALU.mult)
                    cp = pcp.tile([65, 512], F32, tag="cp")
                    for kt in range(KT):
                        nc.tensor.matmul(cp[:, :n], vaug[:, kt, h * 65:(h + 1) * 65],
                                         expm[:, kt, o:o + n],
                                         start=(kt == 0), stop=(kt == KT - 1))
                    # PSUM -> SBUF copies alternate engines to balance ACT/DVE
                    if h % 2 == 0:
                        nc.scalar.copy(ctx_h[:, o:o + n], cp[0:64, :n])
                        nc.scalar.copy(lnd[:, o:o + n], cp[64:65, :n])
                    else:
                        nc.vector.tensor_copy(ctx_h[:, o:o + n], cp[0:64, :n])
                        nc.vector.tensor_copy(lnd[:, o:o + n], cp[64:65, :n])
                nc.sync.dma_start(scr.ap(), lnd[:])
                rq = stream.tile([128, QT], F32, tag="rq")
                nc.sync.dma_start(
                    rq[:], scr.ap().rearrange("(a b) -> b a", b=128))
                nc.vector.reciprocal(rq[:], rq[:])
                for qt in range(QT):
                    qs = slice(qt * 128, (qt + 1) * 128)
                    ps = pmm.tile([128, 512], F32, tag="mm")
                    nc.tensor.matmul(ps[:], ctx_h[:, qs], wo_t[:, h, :],
                                     start=True, stop=True)
                    if h == 0:
                        nc.vector.tensor_scalar(att[:, qt, :], ps[:],
                                                rq[:, qt:qt + 1], None,
                                                op0=ALU.mult)
                    else:
                        nc.vector.scalar_tensor_tensor(
                            att[:, qt, :], ps[:], rq[:, qt:qt + 1],
                            att[:, qt, :], op0=ALU.mult, op1=ALU.add)

            prev = list(expm_pre)
            for h in range(2, H):
                e = emit_scores(h)
                emit_tail(h - 2, prev.pop(0))
                prev.append(e)
            emit_tail(H - 2, prev.pop(0))
            emit_tail(H - 1, prev.pop(0))
            nc.leave_named_scope("p_attn", scope_att[0], False)

            # ---- LayerNorm (ln_g/ln_b applied on the host afterwards):
            # residual add on GpSimd, moments on Vector, center+scale on
            # Scalar (per-partition bias/scale are ACT-legal) ----
            scope_ln = nc.enter_named_scope("p_ln", False)
            varsum = small.tile([128, QT], F32, tag="varsum")
            rstd = small.tile([128, QT], F32, tag="rstd")
            for qt in range(QT):
                xqs = stream.tile([128, D], F32, tag="xqs")
                nc.sync.dma_start(xqs[:], xq_d.ap()[:, qt, :])
                y = att[:, qt, :]
                # y = att + xq (residual) on GpSimd (plain tensor_tensor)
                nc.gpsimd.tensor_tensor(y, y, xqs[:], op=ALU.add)
                musum = stream.tile([128, 1], F32, tag="musum")
                muneg = stream.tile([128, 1], F32, tag="muneg")
                nc.vector.tensor_reduce(musum[:], y, axis=mybir.AxisListType.X,
                                        op=ALU.add)
                nc.vector.tensor_scalar_mul(muneg[:], musum[:], -1.0 / D)
                # yc = y - mu on Scalar; variance via accumulating square
                yc = stream.tile([128, D], F32, tag="yc")
                nc.scalar.activation(yc[:], y, AF.Identity, bias=muneg[:])
                nc.vector.scalar_tensor_tensor(
                    y, yc[:], 1.0, yc[:], op0=ALU.mult, op1=ALU.mult,
                    accum_out=varsum[:, qt:qt + 1])
                # rstd = 1/sqrt(var + eps); z = yc * rstd
                nc.scalar.activation(rstd[:, qt:qt + 1], varsum[:, qt:qt + 1],
                                     AF.Sqrt, scale=1.0 / D, bias=eps_col[:])
                nc.vector.reciprocal(rstd[:, qt:qt + 1], rstd[:, qt:qt + 1])
                z = stream.tile([128, D], F32, tag="z")
                nc.scalar.activation(z[:], yc[:], AF.Identity,
                                     scale=rstd[:, qt:qt + 1])
                nc.sync.dma_start(out_d.ap()[:, qt, :], z[:])
            nc.leave_named_scope("p_ln", scope_ln[0], False)
    nc.compile()
    return nc


def _prep_core(xb, validb, half, perm_k, KV, VT):
    """Host-side shard prep for one core. Returns (in_map, perm_q, xq)."""
    rows = np.arange(half * NQ, (half + 1) * NQ)
    vr = rows[validb[rows]]
    ir = rows[~validb[rows]]
    perm_q = np.concatenate([vr, ir])
    Vq = len(vr)
    Kv = len(perm_k)

    xq = np.ascontiguousarray(xb[perm_q]).astype(np.float32)          # [NQ, D]
    xk = np.zeros((KV, D), np.float32)
    xk[:Kv] = xb[perm_k]
    validk = np.zeros(KV, np.float32)
    validk[:Kv] = 1.0
    nrmk = xk / np.maximum(np.linalg.norm(xk, axis=1, keepdims=True), 1e-12)

    m = {}
    m["xq_t"] = _chunk3(np.ascontiguousarray(xq.T))                   # [128,DC,NQ]
    m["nrmk_t"] = _chunk3(np.ascontiguousarray(nrmk.T.astype(np.float32)))
    m["xq_tb"] = m["xq_t"].astype(ml_dtypes.bfloat16)
    m["xk_tb"] = _chunk3(np.ascontiguousarray(xk.T)).astype(ml_dtypes.bfloat16)
    m["validk_b"] = validk[None, :].astype(ml_dtypes.bfloat16)
    m["simbias_b"] = (-1.0e9 * (1.0 - validk))[None, :].astype(ml_dtypes.bfloat16)
    iq = np.zeros((VT * 128,), np.float32)
    iq[Vq:] = 1.0
    m["invq"] = np.ascontiguousarray(iq.reshape(VT, 128).T)
    return m, perm_q, xq


def kernel(stock_features, stock_valid_mask, in_proj_w, in_proj_b,
           out_w, out_b, ln_g, ln_b):
    x = np.asarray(stock_features, np.float32)
    valid = np.asarray(stock_valid_mask).astype(bool)
    W = np.asarray(in_proj_w, np.float32)
    bqkv = np.asarray(in_proj_b, np.float32)
    Wo = np.asarray(out_w, np.float32)
    bo = np.asarray(out_b, np.float32)
    g = np.asarray(ln_g, np.float32)
    be = np.asarray(ln_b, np.float32)

    perm_ks = [np.where(valid[b])[0] for b in range(B)]
    KV = int(math.ceil(max(len(p) for p in perm_ks) / 128.0)) * 128
    Vq_max = max(
        int(valid[b, half * NQ:(half + 1) * NQ].sum())
        for b in range(B) for half in range(2))
    VT = int(math.ceil(Vq_max / 128.0))

    Wq, Wk, Wv = W[:D], W[D:2 * D], W[2 * D:]
    bq, bk, bv = bqkv[:D], bqkv[D:2 * D], bqkv[2 * D:]
    wv_aug = np.zeros((D, H * 65), np.float32)
    rhs_aug = np.zeros((1, H * 65), np.float32)
    for h in range(H):
        wv_aug[:, h * 65:h * 65 + 64] = Wv.T[:, h * 64:(h + 1) * 64]
        rhs_aug[0, h * 65:h * 65 + 64] = bv[h * 64:(h + 1) * 64]
        rhs_aug[0, h * 65 + 64] = 1.0
    shared = {
        "wq_t": _chunk3(np.ascontiguousarray(Wq.T)).astype(ml_dtypes.bfloat16),
        "wk_t": _chunk3(np.ascontiguousarray(Wk.T)).astype(ml_dtypes.bfloat16),
        "wv_t": _chunk3(wv_aug).astype(ml_dtypes.bfloat16),
        "wo_t": np.ascontiguousarray(
            Wo.T.reshape(H, 64, D).transpose(1, 0, 2)).astype(ml_dtypes.bfloat16),
        "extra_rhs": rhs_aug.astype(ml_dtypes.bfloat16),
        "onesb": np.ones((1, 128), ml_dtypes.bfloat16),
        "identb": np.eye(128, dtype=ml_dtypes.bfloat16),
        "bq_col": np.ascontiguousarray(bq.reshape(DC, 128).T),
        "bk_col": np.ascontiguousarray(bk.reshape(DC, 128).T),
    }

    in_maps = []
    perms = []
    for b in range(B):
        for half in range(2):
            m, perm_q, xq = _prep_core(x[b], valid[b], half, perm_ks[b], KV, VT)
            m.update(shared)
            m["xq"] = np.ascontiguousarray(
                (xq + bo[None, :]).reshape(QT, 128, D).transpose(1, 0, 2))
            in_maps.append(m)
            perms.append((b, perm_q))

    KS = int(math.ceil(max(len(p) for p in perm_ks) / 8.0)) * 8
    nc = build_nc(KV, VT, KS)
    res = bass_utils.run_bass_kernel_spmd(nc, in_maps, core_ids=list(range(8)))

    out = np.zeros((B, N, D), np.float32)
    for core, (b, perm_q) in enumerate(perms):
        o = np.asarray(res.results[core]["out"])      # [128, QT, D]
        out[b, perm_q] = o.transpose(1, 0, 2).reshape(NQ, D)
    # LayerNorm affine applied host-side (device outputs the normalized y)
    return out * g[None, None, :] + be[None, None, :]


# revision 58
# speedup vs baseline: 1.1679x; 1.1679x over previous
"""Trainium2 Bass kernel for CrossStockAttention (sparse top-40 attention).

Strategy (8 NeuronCores, zero inter-core communication):
  - Data-parallel: core = (batch b, query-half). Each core owns 1024 queries of
    one batch and all of that batch's valid keys (compacted, padded to 128).
  - Host-side marshalling: queries permuted valid-first; keys compacted to
    valid-only. Permutation undone on the host after the gather.
  - Ranking trick: cosine top-k per query row is invariant to the positive
    per-row scale 1/|x_q|, so only KEYS are normalized (sim = x_q . x_k/|x_k|).
  - Exact top-40 threshold per valid query: 5 rounds of DVE max8+match_replace
    give the 40 largest values; t40 = min of the last round's 8. The mask is
    (sim >= t40) -- exact fp32, tie-free for random data. Invalid-query rows
    get t40 = -1e30 (attend to every valid key; padded keys are harmless since
    their V rows and denominator gates are zero).
  - Attention in transposed score layout S_T[k, q]: the softmax denominator is
    an extra all-ones column appended to V (gated by the valid-key indicator),
    and attn@V needs no transposes.
  - Division by the softmax denominator is folded into the per-head output
    projection merge via scalar_tensor_tensor in token-major layout.
  - Engine balance: LayerNorm runs on the (otherwise idle) GpSimd engine; exp
    on Scalar; top-k scan on Vector; matmuls dense on PE. Scores+exp for heads
    0-1 are emitted during the top-k phase so PE/ACT overlap the DVE scan.
"""

import math
import numpy as np
import ml_dtypes

import concourse.bass as bass
import concourse.mybir as mybir
from concourse.tile import TileContext
from concourse import bass_utils, bacc

B, N, D, H, TOPK = 4, 2048, 512, 8, 40
DH = D // H
NQ = N // 2          # queries per core
QT = NQ // 128       # query tiles (8)
DC = D // 128        # feature chunks (4)
F32 = mybir.dt.float32
BF16 = mybir.dt.bfloat16
AF = mybir.ActivationFunctionType
ALU = mybir.AluOpType


def _chunk3(x, p=128):
    """[A*p, F] -> [p, A, F] (partition-major chunking along dim0)."""
    a = x.shape[0] // p
    return np.ascontiguousarray(x.reshape(a, p, -1).transpose(1, 0, 2))


def _nchunks(total, step=512):
    out = []
    o = 0
    while o < total:
        out.append((o, min(step, total - o)))
        o += step
    return out


def build_nc(KV, VT, KS):
    """Build the single-core program (SPMD across 8 cores, data differs).

    KS = number of key columns that can hold real keys (max valid count,
    rounded up to 8). Columns beyond KS are -1e9 bias with zero V rows, so
    the top-k scan and the sim products skip them.
    """
    KT = KV // 128       # key tiles
    NV = VT * 128        # padded valid-query count
    kv_ch = _nchunks(KV)
    v_ch = _nchunks(H * 65)

    nc = bacc.Bacc("TRN2", target_bir_lowering=False, debug=False, num_devices=8)

    di = lambda name, shape: nc.dram_tensor(name, shape, F32, kind="ExternalInput")
    db = lambda name, shape: nc.dram_tensor(name, shape, BF16, kind="ExternalInput")
    # group 1: gates the projections (small, bf16) -- loaded first
    wq_t_d = db("wq_t", [128, DC, D])
    bq_col_d = di("bq_col", [128, DC])
    bk_col_d = di("bk_col", [128, DC])
    xq_tb_d = db("xq_tb", [128, DC, NQ])
    wk_t_d = db("wk_t", [128, DC, D])
    wv_t_d = db("wv_t", [128, DC, H * 65])
    xk_tb_d = db("xk_tb", [128, DC, KV])
    validk_b_d = db("validk_b", [1, KV])
    extra_rhs_d = db("extra_rhs", [1, H * 65])
    # group 2: gates sim + topk (nrmk = x_k/|x_k| precomputed on host).
    # sim MUST be fp32: the top-40 boundary decisions have gaps down to
    # ~1e-6 and must match the fp32 reference.
    nrmk_t_d = di("nrmk_t", [128, DC, KV])
    xq_t_d = di("xq_t", [128, DC, NQ])
    simbias_b_d = db("simbias_b", [1, KV])
    onesb_d = db("onesb", [1, 128])
    invq_d = di("invq", [128, VT])
    identb_d = db("identb", [128, 128])
    # group 3: needed late (attention tail / LN)
    wo_t_d = db("wo_t", [64, H, D])
    xq_d = di("xq", [128, QT, D])
    lnd_scratch = [nc.dram_tensor(f"lnd_scratch{i}", [NQ], F32, kind="Internal")
                   for i in range(2)]
    out_d = nc.dram_tensor("out", [128, QT, D], F32, kind="ExternalOutput")

    with TileContext(nc) as tc:
        with (
            tc.tile_pool(name="consts", bufs=1) as consts,
            tc.tile_pool(name="bigbuf", bufs=1) as bigbuf,
            tc.tile_pool(name="stream", bufs=2) as stream,
            tc.tile_pool(name="simp", bufs=2) as simp,
            tc.tile_pool(name="pmm", bufs=2, space="PSUM") as pmm,
            tc.tile_pool(name="psc", bufs=2, space="PSUM") as psc,
            tc.tile_pool(name="pcp", bufs=2, space="PSUM") as pcp,
            tc.tile_pool(name="small", bufs=1) as small,
        ):
            def load(dram, shape, dtype=F32, pool=consts, tag=None):
                t = pool.tile(shape, dtype, tag=tag or dram.name)
                nc.sync.dma_start(t[:], dram.ap())
                return t

            def load_chunked(dram, shape, dtype, tag, pool=bigbuf):
                t = pool.tile(shape, dtype, tag=tag)
                for c in range(shape[1]):
                    nc.sync.dma_start(t[:, c, :], dram.ap()[:, c, :])
                return t

            # ---- loads, ordered by first use: Q-proj -> sim -> K/V proj ----
            wq_t = load(wq_t_d, [128, DC, D], BF16)
            bq_col = load(bq_col_d, [128, DC])
            bk_col = load(bk_col_d, [128, DC])
            xq_tb = load_chunked(xq_tb_d, [128, DC, NQ], BF16, "xq_tb")
            nrmk_t = load_chunked(nrmk_t_d, [128, DC, KV], F32, "nrmk")
            xq_t = load_chunked(xq_t_d, [128, DC, NQ], F32, "xq_t")
            simbias_b = load(simbias_b_d, [1, KV], BF16)
            onesb = load(onesb_d, [1, 128], BF16)
            invq = load(invq_d, [128, VT])
            identb = load(identb_d, [128, 128], BF16)
            wk_t = load(wk_t_d, [128, DC, D], BF16)
            xk_tb = load_chunked(xk_tb_d, [128, DC, KV], BF16, "xk_tb")
            wv_t = load(wv_t_d, [128, DC, H * 65], BF16)
            validk_b = load(validk_b_d, [1, KV], BF16)
            extra_rhs = load(extra_rhs_d, [1, H * 65], BF16)
            wo_t = load(wo_t_d, [64, H, D], BF16)

            eps_col = consts.tile([128, 1], F32)
            nc.vector.memset(eps_col[:], 1.0e-5)

            # ---- attention scores (emitted per head; h0/h1 overlap top-k) ----
            att = bigbuf.tile([128, QT, D], F32, tag="att")

            def emit_scores(h):
                hp = (h % 2) * 64
                hc = h // 2
                # three slots deepen the scores->exp->tail pipeline; slot
                # "nrmk" is the sim-input buffer, dead once head 2 starts
                expm = bigbuf.tile([128, KT, NQ], BF16,
                                   tag=["expm0", "expm1", "nrmk"][h % 3])
                for kt in range(KT):
                    ks = slice(kt * 128, (kt + 1) * 128)
                    ps = psc.tile([128, 1024], F32, tag="sc")
                    for (o, n) in _nchunks(NQ):
                        nc.tensor.matmul(
                            ps[:, o:o + n],
                            kt_sb[hp:hp + 64, hc, ks],
                            qt_sb[hp:hp + 64, hc, o:o + n],
                            start=True, stop=True)
                    nc.scalar.activation(expm[:, kt, :], ps[:],
                                         AF.Exp, scale=1.0 / math.sqrt(DH))
                return expm

            # ---- sim (bf16 hi/lo) + exact top-40 threshold mask per q tile ----
            scope_sim = nc.enter_named_scope("p_simtopk", False)
            maskT = bigbuf.tile([128, KT, NV], BF16, tag="maskT")
            work = small.tile([128, KV], F32, tag="work")
            scr8 = small.tile([128, 8], F32, tag="scr8")
            t40 = small.tile([128, 1], F32, tag="t40")

            def emit_sim_topk(vt):
                qs = slice(vt * 128, (vt + 1) * 128)
                sim = simp.tile([128, KV], F32, tag="sim")
                for (o, n) in kv_ch:
                    ps = pmm.tile([128, 512], F32, tag="mm")
                    nc.tensor.matmul(ps[:, :n], onesb[:],
                                     simbias_b[:, o:o + n], start=True, stop=False)
                    nr = max(0, min(KS - o, n))  # cols that can hold real keys
                    for c in range(DC):
                        nc.tensor.matmul(ps[:, :nr], xq_t[:, c, qs],
                                         nrmk_t[:, c, o:o + nr],
                                         start=False, stop=(c == DC - 1))
                    nc.scalar.copy(sim[:, o:o + n], ps[:, :n])
                # round 1 reads sim directly; match_replace writes the work
                # copy; the final round needs no replace (only its max8)
                nc.vector.max(scr8[:], sim[:, 0:KS])
                nc.vector.match_replace(work[:, 0:KS], scr8[:], sim[:, 0:KS],
                                        -1.0e9)
                for r in range(TOPK // 8 - 2):
                    nc.vector.max(scr8[:], work[:, 0:KS])
                    nc.vector.match_replace(work[:, 0:KS], scr8[:],
                                            work[:, 0:KS], -1.0e9)
                nc.vector.max(scr8[:], work[:, 0:KS])
                # t40 = 40th-largest; invalid-query rows attend to everything
                nc.vector.tensor_reduce(t40[:], scr8[:], axis=mybir.AxisListType.X,
                                        op=ALU.min)
                nc.vector.scalar_tensor_tensor(
                    t40[:], invq[:, vt:vt + 1], -1.0e30, t40[:],
                    op0=ALU.mult, op1=ALU.add)
                maskv = simp.tile([128, KV], BF16, tag="maskv")
                nc.vector.tensor_scalar(maskv[:], sim[:], t40[:], None,
                                        op0=ALU.is_ge)
                # transpose this tile's mask into maskT[k, q]
                for kt in range(KT):
                    pt = pcp.tile([128, 128], BF16, tag="cp")
                    nc.tensor.transpose(
                        pt[:], maskv[:, kt * 128:(kt + 1) * 128], identb[:])
                    nc.vector.tensor_copy(maskT[:, kt, qs], pt[:])

            # ---- Q projection first (its inputs land first), then the sim
            # tiles that gate the serial Vector top-k chain ----
            qt_sb = bigbuf.tile([128, DC, NQ], BF16, tag="qt")
            for dot in range(DC):
                for (o, n) in _nchunks(NQ):
                    ps = pmm.tile([128, 512], F32, tag="mm")
                    for c in range(DC):
                        nc.tensor.matmul(
                            ps[:, :n],
                            wq_t[:, c, dot * 128:(dot + 1) * 128],
                            xq_tb[:, c, o:o + n],
                            start=(c == 0), stop=(c == DC - 1))
                    nc.scalar.activation(qt_sb[:, dot, o:o + n], ps[:, :n],
                                         AF.Identity, bias=bq_col[:, dot:dot + 1])

            emit_sim_topk(0)
            emit_sim_topk(1)

            kt_sb = bigbuf.tile([128, DC, KV], BF16, tag="kt")
            for dot in range(DC):
                for (o, n) in kv_ch:
                    ps = pmm.tile([128, 512], F32, tag="mm")
                    for c in range(DC):
                        nc.tensor.matmul(
                            ps[:, :n],
                            wk_t[:, c, dot * 128:(dot + 1) * 128],
                            xk_tb[:, c, o:o + n],
                            start=(c == 0), stop=(c == DC - 1))
                    nc.scalar.activation(kt_sb[:, dot, o:o + n], ps[:, :n],
                                         AF.Identity, bias=bk_col[:, dot:dot + 1])

            emit_sim_topk(2)
            emit_sim_topk(3)
            emit_sim_topk(4)
            expm_pre = [emit_scores(0)]

            vaug = bigbuf.tile([128, KT, H * 65], BF16, tag="vaug")
            for kt in range(KT):
                ks = slice(kt * 128, (kt + 1) * 128)
                for (o, n) in v_ch:
                    ps = pmm.tile([128, 512], F32, tag="mm")
                    nc.tensor.matmul(ps[:, :n], validk_b[0:1, ks],
                                     extra_rhs[:, o:o + n], start=True, stop=False)
                    for c in range(DC):
                        nc.tensor.matmul(
                            ps[:, :n], xk_tb[:, c, ks],
                            wv_t[:, c, o:o + n],
                            start=False, stop=(c == DC - 1))
                    nc.scalar.copy(vaug[:, kt, o:o + n], ps[:, :n])

            expm_pre.append(emit_scores(1))
            nc.leave_named_scope("p_simtopk", scope_sim[0], False)

            # ---- attention tails, software-pipelined across heads ----
            scope_att = nc.enter_named_scope("p_attn", False)

            def emit_tail(h, expm):
                # mask multiply: GpSimd takes some heads to relieve Vector.
                # Head 0 splits by ctx chunk on GpSimd so its first ctx
                # matmul starts while Vector is still scanning sim tile 4.
                eng = nc.gpsimd if h in (0, 5, 6, 7) else nc.vector
                if h != 0:
                    for kt in range(KT):
                        eng.tensor_tensor(expm[:, kt, 0:NV], expm[:, kt, 0:NV],
                                          maskT[:, kt, :], op=ALU.mult)
                ctx_h = stream.tile([64, NQ], BF16, tag="ctx")
                lnd = small.tile([1, NQ], F32, tag="lnd")
                scr = lnd_scratch[h % 2]
                for (o, n) in _nchunks(NQ):
                    if h == 0:
                        mo, mn = o, max(0, min(NV, o + n) - o)
                        for kt in range(KT):
                            eng.tensor_tensor(expm[:, kt, mo:mo + mn],
                                              expm[:, kt, mo:mo + mn],
                                              maskT[:, kt, mo:mo + mn],
                                              op=ALU.mult)
                    cp = pcp.tile([65, 512], F32, tag="cp")
                    for kt in range(KT):
                        nc.tensor.matmul(cp[:, :n], vaug[:, kt, h * 65:(h + 1) * 65],
                                         expm[:, kt, o:o + n],
                                         start=(kt == 0), stop=(kt == KT - 1))
                    # PSUM -> SBUF copies alternate engines to balance ACT/DVE
                    if h % 2 == 0:
                        nc.scalar.copy(ctx_h[:, o:o + n], cp[0:64, :n])
                        nc.scalar.copy(lnd[:, o:o + n], cp[64:65, :n])
                    else:
                        nc.vector.tensor_copy(ctx_h[:, o:o + n], cp[0:64, :n])
                        nc.vector.tensor_copy(lnd[:, o:o + n], cp[64:65, :n])
                nc.sync.dma_start(scr.ap(), lnd[:])
                rq = stream.tile([128, QT], F32, tag="rq")
                nc.sync.dma_start(
                    rq[:], scr.ap().rearrange("(a b) -> b a", b=128))
                nc.vector.reciprocal(rq[:], rq[:])
                for qt in range(QT):
                    qs = slice(qt * 128, (qt + 1) * 128)
                    ps = pmm.tile([128, 512], F32, tag="mm")
                    nc.tensor.matmul(ps[:], ctx_h[:, qs], wo_t[:, h, :],
                                     start=True, stop=True)
                    if h == 0:
                        nc.vector.tensor_scalar(att[:, qt, :], ps[:],
                                                rq[:, qt:qt + 1], None,
                                                op0=ALU.mult)
                    else:
                        nc.vector.scalar_tensor_tensor(
                            att[:, qt, :], ps[:], rq[:, qt:qt + 1],
                            att[:, qt, :], op0=ALU.mult, op1=ALU.add)

            prev = list(expm_pre)
            for h in range(2, H):
                e = emit_scores(h)
                emit_tail(h - 2, prev.pop(0))
                prev.append(e)
            emit_tail(H - 2, prev.pop(0))
            emit_tail(H - 1, prev.pop(0))
            nc.leave_named_scope("p_attn", scope_att[0], False)

            # ---- LayerNorm (ln_g/ln_b applied on the host afterwards):
            # residual add on GpSimd, moments on Vector, center+scale on
            # Scalar (per-partition bias/scale are ACT-legal) ----
            scope_ln = nc.enter_named_scope("p_ln", False)
            musum = small.tile([128, 1], F32, tag="musum")
            muneg = small.tile([128, 1], F32, tag="muneg")
            varsum = small.tile([128, QT], F32, tag="varsum")
            rstd = small.tile([128, QT], F32, tag="rstd")
            for qt in range(QT):
                xqs = stream.tile([128, D], F32, tag="xqs")
                nc.sync.dma_start(xqs[:], xq_d.ap()[:, qt, :])
                y = att[:, qt, :]
                # y = att + xq (residual) on GpSimd (plain tensor_tensor)
                nc.gpsimd.tensor_tensor(y, y, xqs[:], op=ALU.add)
                nc.vector.tensor_reduce(musum[:], y, axis=mybir.AxisListType.X,
                                        op=ALU.add)
                nc.vector.tensor_scalar_mul(muneg[:], musum[:], -1.0 / D)
                # yc = y - mu on Scalar; variance via accumulating square
                yc = stream.tile([128, D], F32, tag="yc")
                nc.scalar.activation(yc[:], y, AF.Identity, bias=muneg[:])
                nc.vector.scalar_tensor_tensor(
                    y, yc[:], 1.0, yc[:], op0=ALU.mult, op1=ALU.mult,
                    accum_out=varsum[:, qt:qt + 1])
                # rstd = 1/sqrt(var + eps); z = yc * rstd
                nc.scalar.activation(rstd[:, qt:qt + 1], varsum[:, qt:qt + 1],
                                     AF.Sqrt, scale=1.0 / D, bias=eps_col[:])
                nc.vector.reciprocal(rstd[:, qt:qt + 1], rstd[:, qt:qt + 1])
                z = stream.tile([128, D], F32, tag="z")
                nc.scalar.activation(z[:], yc[:], AF.Identity,
                                     scale=rstd[:, qt:qt + 1])
                nc.sync.dma_start(out_d.ap()[:, qt, :], z[:])
            nc.leave_named_scope("p_ln", scope_ln[0], False)
    nc.compile()
    return nc


def _prep_core(xb, validb, half, perm_k, KV, VT):
    """Host-side shard prep for one core. Returns (in_map, perm_q, xq)."""
    rows = np.arange(half * NQ, (half + 1) * NQ)
    vr = rows[validb[rows]]
    ir = rows[~validb[rows]]
    perm_q = np.concatenate([vr, ir])
    Vq = len(vr)
    Kv = len(perm_k)

    xq = np.ascontiguousarray(xb[perm_q]).astype(np.float32)          # [NQ, D]
    xk = np.zeros((KV, D), np.float32)
    xk[:Kv] = xb[perm_k]
    validk = np.zeros(KV, np.float32)
    validk[:Kv] = 1.0
    nrmk = xk / np.maximum(np.linalg.norm(xk, axis=1, keepdims=True), 1e-12)

    m = {}
    m["xq_t"] = _chunk3(np.ascontiguousarray(xq.T))                   # [128,DC,NQ]
    m["nrmk_t"] = _chunk3(np.ascontiguousarray(nrmk.T.astype(np.float32)))
    m["xq_tb"] = m["xq_t"].astype(ml_dtypes.bfloat16)
    m["xk_tb"] = _chunk3(np.ascontiguousarray(xk.T)).astype(ml_dtypes.bfloat16)
    m["validk_b"] = validk[None, :].astype(ml_dtypes.bfloat16)
    m["simbias_b"] = (-1.0e9 * (1.0 - validk))[None, :].astype(ml_dtypes.bfloat16)
    iq = np.zeros((VT * 128,), np.float32)
    iq[Vq:] = 1.0
    m["invq"] = np.ascontiguousarray(iq.reshape(VT, 128).T)
    return m, perm_q, xq


def kernel(stock_features, stock_valid_mask, in_proj_w, in_proj_b,
           out_w, out_b, ln_g, ln_b):
    x = np.asarray(stock_features, np.float32)
    valid = np.asarray(stock_valid_mask).astype(bool)
    W = np.asarray(in_proj_w, np.float32)
    bqkv = np.asarray(in_proj_b, np.float32)
    Wo = np.asarray(out_w, np.float32)
    bo = np.asarray(out_b, np.float32)
    g = np.asarray(ln_g, np.float32)
    be = np.asarray(ln_b, np.float32)

    perm_ks = [np.where(valid[b])[0] for b in range(B)]
    KV = int(math.ceil(max(len(p) for p in perm_ks) / 128.0)) * 128
    Vq_max = max(
        int(valid[b, half * NQ:(half + 1) * NQ].sum())
        for b in range(B) for half in range(2))
    VT = int(math.ceil(Vq_max / 128.0))

    Wq, Wk, Wv = W[:D], W[D:2 * D], W[2 * D:]
    bq, bk, bv = bqkv[:D], bqkv[D:2 * D], bqkv[2 * D:]
    wv_aug = np.zeros((D, H * 65), np.float32)
    rhs_aug = np.zeros((1, H * 65), np.float32)
    for h in range(H):
        wv_aug[:, h * 65:h * 65 + 64] = Wv.T[:, h * 64:(h + 1) * 64]
        rhs_aug[0, h * 65:h * 65 + 64] = bv[h * 64:(h + 1) * 64]
        rhs_aug[0, h * 65 + 64] = 1.0
    shared = {
        "wq_t": _chunk3(np.ascontiguousarray(Wq.T)).astype(ml_dtypes.bfloat16),
        "wk_t": _chunk3(np.ascontiguousarray(Wk.T)).astype(ml_dtypes.bfloat16),
        "wv_t": _chunk3(wv_aug).astype(ml_dtypes.bfloat16),
        "wo_t": np.ascontiguousarray(
            Wo.T.reshape(H, 64, D).transpose(1, 0, 2)).astype(ml_dtypes.bfloat16),
        "extra_rhs": rhs_aug.astype(ml_dtypes.bfloat16),
        "onesb": np.ones((1, 128), ml_dtypes.bfloat16),
        "identb": np.eye(128, dtype=ml_dtypes.bfloat16),
        "bq_col": np.ascontiguousarray(bq.reshape(DC, 128).T),
        "bk_col": np.ascontiguousarray(bk.reshape(DC, 128).T),
    }

    in_maps = []
    perms = []
    for b in range(B):
        for half in range(2):
            m, perm_q, xq = _prep_core(x[b], valid[b], half, perm_ks[b], KV, VT)
            m.update(shared)
            m["xq"] = np.ascontiguousarray(
                (xq + bo[None, :]).reshape(QT, 128, D).transpose(1, 0, 2))
            in_maps.append(m)
            perms.append((b, perm_q))

    KS = int(math.ceil(max(len(p) for p in perm_ks) / 8.0)) * 8
    nc = build_nc(KV, VT, KS)
    res = bass_utils.run_bass_kernel_spmd(nc, in_maps, core_ids=list(range(8)))

    out = np.zeros((B, N, D), np.float32)
    for core, (b, perm_q) in enumerate(perms):
        o = np.asarray(res.results[core]["out"])      # [128, QT, D]
        out[b, perm_q] = o.transpose(1, 0, 2).reshape(NQ, D)
    # LayerNorm affine applied host-side (device outputs the normalized y)
    return out * g[None, None, :] + be[None, None, :]
